# revision 34
# baseline (speedup 1.0000x reference)
"""Causal self-attention (GQA + RoPE) for TRN2, sharded over 8 NeuronCores.

Sharding: tensor-parallel over heads. Each core owns 4 query heads and 1 KV
head (H=32, HKV=8 -> group size 4). Column-parallel q/k/v projections,
row-parallel o_proj; the final all-reduce over the 8 partial [T, D] outputs
happens on the host after the gather.

Performance design (377.8us f32r baseline -> 214.4us):
  - All matmul operands are bf16 (PSUM accumulation stays fp32): halves
    every DMA and LDWEIGHTS. PSUM f32 throughout; rel err 4.4e-3 vs 2e-2
    tolerance.
  - Software-pipelined emission with lookahead 2: scores for chunks j+1,
    j+2 are issued to the in-order PE queue BEFORE attn@v of chunk j, so
    the PE never head-of-line blocks on the ACT exp of the current chunk
    (the baseline idled the PE ~50% through every attention phase, which
    also dropped the PE p-state clock from 2.4 to 1.2 GHz).
  - Projection of strip s+1 and o_proj of strip s-1 ride as PE gap fillers
    inside strip s's attention, gated to the last 2/3 of the strip so they
    never wait on in-flight input DMAs; half of the second-to-last o_proj
    is deferred into the final strip, whose attention is otherwise
    ACT-bound.
  - PSUM pools are split by lifetime class (scores / projection
    accumulators / o_proj+transpose+broadcast / attn accumulators =
    2+2+2+2 banks) so fast-churning score tiles never recycle behind a
    16-matmul projection group. In the final strip (no projection
    fillers) the scores alternate between the two rings, doubling the
    effective ring depth and unchaining the chunk period from the exp
    latency.
  - Softmax reciprocal runs on the DVE (reciprocal_approx_fast custom op,
    full-tile form -- the 1-row slice form mis-executes) so ACT keeps its
    Exp table loaded for the whole kernel (the baseline burned 41us in
    ACT_TABLE_LOAD ping-pong); the 1/denom row is broadcast across
    partitions by a K=1 PE outer product.
  - o_proj eviction copies run on the DVE (final strip: alternating
    DVE/ACT), writing bf16 into a packed [128, D] row buffer; stores are
    one DMA per (strip, tsub), split across two queues at the tail.
  - Startup loads are split across the sync/scalar/gpsimd queues (SWDGE
    descriptor generation is 3-7us per big rearranged DMA and serializes
    per queue), ordered so the first projection chunk lands in ~5us; x
    strips load as one 3D-rearranged DMA per strip, double buffered.
Layout (all on-chip tensors keep the contraction dim on partitions):
  - x is uploaded pre-transposed as xT [D, T]; projections produce qT/kT/vT
    [d, t] directly in PSUM.
  - RoPE uses a host-side permutation of the head dim into [even-pairs |
    odd-pairs] halves so the pair rotation becomes: out = q*cos +
    swap64(q*sin'), where swap64 swaps 32-row halves within each 64-row
    head block (SBUF->SBUF DMAs; q on sync queue, k on gpsimd) and sin'
    carries the sign pattern [+sin | -sin].
  - Scores are computed transposed (scoresT [s, t]) so the softmax
    denominator comes out of the attn@v matmul for free via the ones
    column in the v stationary operand; causal masking of diagonal blocks
    is a post-exp gpsimd affine_select.
"""

import math

import numpy as np

import concourse.bass as bass
import concourse.mybir as mybir
import concourse.tile as tile
from concourse import bacc
from concourse.masks import make_identity

D = 2048
H = 32
HKV = 8
HD = 64
T = 2048
NCORES = 8
HPC = H // NCORES        # 4 query heads per core
QC = HPC * HD            # 256 q dims per core
ROPE_BASE = 10000.0
S = 512                  # t-strip / moving-operand width
NSTRIP = T // S          # 4
KC = D // 128            # 16 contraction chunks

F32 = mybir.dt.float32
BF16 = mybir.dt.bfloat16


def _build_kernel(debug=False):
    nc = bacc.Bacc("TRN2", target_bir_lowering=False, debug=False,
                   num_devices=NCORES)

    xT = nc.dram_tensor("xT", [D, T], BF16, kind="ExternalInput").ap()
    wqT = nc.dram_tensor("wqT", [D, QC], BF16, kind="ExternalInput").ap()
    wkvT = nc.dram_tensor("wkvT", [D, 128], BF16, kind="ExternalInput").ap()
    woT = nc.dram_tensor("woT", [QC, D], BF16, kind="ExternalInput").ap()
    cosT = nc.dram_tensor("cosT", [128, T], BF16,
                          kind="ExternalInput").ap()
    sinT = nc.dram_tensor("sinT", [128, T], BF16,
                          kind="ExternalInput").ap()
    out = nc.dram_tensor("out", [T, D], BF16, kind="ExternalOutput").ap()
    dbg = {}
    if debug:
        for nm, shp, dt in [("d_qT0", [128, T], BF16), ("d_qT1", [128, T], BF16),
                            ("d_kT", [128, T], BF16),
                            ("d_vaug", [128, 16 * 65], BF16),
                            ("d_exp", [128, 4 * S], BF16),
                            ("d_yt", [128, S], F32), ("d_dn", [128, S], F32),
                            ("d_bc", [128, S], F32),
                            ("d_ytn0", [128, S], BF16),
                            ("d_ytn1", [128, S], BF16)]:
            dbg[nm] = nc.dram_tensor(nm, shp, dt, kind="ExternalOutput").ap()

    with tile.TileContext(nc) as tc:
        with (
            tc.tile_pool(name="consts", bufs=1) as consts,
            tc.tile_pool(name="persist", bufs=1) as persist,
            tc.tile_pool(name="xa", bufs=3) as xap,
            tc.tile_pool(name="rtmp", bufs=6) as rtmp,
            tc.tile_pool(name="swp", bufs=4) as swp,
            tc.tile_pool(name="vtmp", bufs=2) as vtmp,
            tc.tile_pool(name="expp", bufs=6) as expp,
            tc.tile_pool(name="ytn", bufs=6) as ytnp,
            tc.tile_pool(name="outst", bufs=3) as outst,
            tc.tile_pool(name="dn", bufs=4) as dnp,
            tc.tile_pool(name="mmS", bufs=2, space="PSUM") as mmS,
            tc.tile_pool(name="mmP", bufs=2, space="PSUM") as mmP,
            tc.tile_pool(name="mmO", bufs=2, space="PSUM") as mmO,
            tc.tile_pool(name="ytps", bufs=2, space="PSUM") as ytps,
        ):
            # ---- constants; DMA issue order interleaved per k-chunk so the
            # first projection matmul only waits on chunk 0 of wq/x ----
            wq_sb = consts.tile([128, KC, QC], BF16)
            wkv_sb = consts.tile([128, KC, 128], BF16)
            xa_strips = {}

            def load_xa(strip):
                t0 = strip * S
                xt = xap.tile([128, KC, S], BF16, tag="xa",
                              name=f"xa{strip}")
                nc.sync.dma_start(
                    out=xt,
                    in_=xT.rearrange("(c p) t -> p c t", p=128)[:, :,
                                                               t0:t0 + S])
                xa_strips[strip] = [xt[:, kc, :] for kc in range(KC)]

            wqT_r = wqT.rearrange("(c p) q -> p c q", p=128)
            xT_r = xT.rearrange("(c p) t -> p c t", p=128)
            # startup loads spread across 4 queues so SWDGE descriptor
            # generation (3-7us per big rearranged DMA) runs in parallel
            xt0 = xap.tile([128, KC, S], BF16, tag="xa", name="xa0")
            nc.sync.dma_start(out=xt0[:, 0:4, :], in_=xT_r[:, 0:4, 0:S])
            nc.scalar.dma_start(out=wq_sb[:, 0:4, :], in_=wqT_r[:, 0:4, :])
            nc.sync.dma_start(out=xt0[:, 4:KC, :], in_=xT_r[:, 4:KC, 0:S])
            xa_strips[0] = [xt0[:, kc, :] for kc in range(KC)]
            nc.gpsimd.dma_start(
                out=wkv_sb, in_=wkvT.rearrange("(c p) q -> p c q", p=128))
            nc.scalar.dma_start(out=wq_sb[:, 4:KC, :], in_=wqT_r[:, 4:KC, :])
            cs_c = consts.tile([128, T], BF16)
            cs_s = consts.tile([128, T], BF16)
            nc.scalar.dma_start(out=cs_c, in_=cosT)
            nc.sync.dma_start(out=cs_s, in_=sinT)
            wo_sb = consts.tile([128, 2, D], BF16)
            ident_f = consts.tile([128, 128], F32)
            make_identity(nc, ident_f)
            ident = consts.tile([128, 128], BF16)
            nc.vector.tensor_copy(ident, ident_f)
            # PE warmup: keep the array continuously busy while the first
            # input DMAs land so the p-state ramp is done by the first
            # projection matmul (fp32 on purpose: 4 cyc/row keeps the array
            # busy longer per instruction)
            warm_ps = mmO.tile([128, 512], F32, tag="mmO", name="warm")
            junk = consts.tile([128, 512], F32)
            nc.vector.memset(junk, 1.0)
            for w in range(2):
                nc.tensor.matmul(
                    warm_ps, ident_f, junk,
                    start=True, stop=True, skip_group_check=True)
            ones_b = consts.tile([128, 64], BF16)
            nc.vector.memset(ones_b, 1.0)

            # persistent activations
            qT = [persist.tile([128, T], BF16, tag=f"qT{i}", name=f"qT{i}")
                  for i in range(2)]
            # k duplicated on both partition halves so each q head can use
            # a stationary slice whose base partition matches its rhs base
            kT = persist.tile([128, T], BF16)
            # vaug columns: [v(64) | ones]; the ones column makes the
            # softmax denominator fall out of the attn@v matmul (row 64)
            vaug = persist.tile([128, 4 * NSTRIP, 65], BF16)
            ones_col = consts.tile([128, 4 * NSTRIP, 1], F32)
            nc.vector.memset(ones_col, 1.0)
            nc.vector.tensor_copy(vaug[:, :, 64:65], ones_col)

            def proj_filler(strip, dense=False, part=None):
                """Yield closures, each emitting one PE op of this strip's
                q/kv projection; rope/evict DVE work rides along after the
                last matmul of each accumulation group. The dense (pre-loop)
                call borrows the then-idle mmS ring for the q tiles so the
                three groups are not serialized by the 1-slot mmP ring."""
                qpool, qtag = (mmP, "mmP")
                kvpool, kvtag = (mmS, "mmS") if dense else (mmP, "mmP")
                t0 = strip * S
                tsl = slice(t0, t0 + S)
                xa = xa_strips[strip]

                def rope_q(hp, pq):
                    qc = rtmp.tile([128, S], F32, tag="rtmp",
                                   name=f"qc{strip}{hp}")
                    qs = rtmp.tile([128, S], F32, tag="rtmp",
                                   name=f"qs{strip}{hp}")
                    nc.vector.tensor_mul(qs, pq, cs_s[:, tsl])
                    sw = swp.tile([128, S], F32, tag="swp",
                                  name=f"sw{strip}{hp}")
                    for b in range(2):
                        nc.sync.dma_start(
                            out=sw[b * 64:b * 64 + 32, :],
                            in_=qs[b * 64 + 32:b * 64 + 64, :])
                        nc.sync.dma_start(
                            out=sw[b * 64 + 32:b * 64 + 64, :],
                            in_=qs[b * 64:b * 64 + 32, :])
                    nc.vector.tensor_mul(qc, pq, cs_c[:, tsl])
                    nc.vector.tensor_add(qT[hp][:, tsl], qc, sw)

                if part != "b":
                    pq0 = qpool.tile([128, S], F32, tag=qtag,
                                     name=f"pq{strip}_0")
                    for kc in range(KC):
                        def mk(kc=kc):
                            nc.tensor.matmul(
                                pq0, wq_sb[:, kc, 0:128],
                                xa[kc], start=(kc == 0),
                                stop=(kc == KC - 1))
                            if kc == KC - 1:
                                rope_q(0, pq0)
                        yield mk

                def rope_kv():
                    kc_t = rtmp.tile([128, S], F32, tag="rtmp",
                                     name=f"kc{strip}")
                    ks_t = rtmp.tile([128, S], F32, tag="rtmp",
                                     name=f"ks{strip}")
                    nc.vector.tensor_mul(
                        ks_t[0:64, :], pkv[0:64, :], cs_s[0:64, tsl])
                    swk = swp.tile([128, S], F32, tag="swp",
                                   name=f"swk{strip}")
                    nc.gpsimd.dma_start(out=swk[0:32, :], in_=ks_t[32:64, :])
                    nc.gpsimd.dma_start(out=swk[32:64, :], in_=ks_t[0:32, :])
                    nc.vector.tensor_mul(
                        kc_t[0:64, :], pkv[0:64, :], cs_c[0:64, tsl])
                    nc.vector.tensor_add(
                        kT[0:64, tsl], kc_t[0:64, :], swk[0:64, :])
                    nc.gpsimd.dma_start(out=kT[64:128, tsl], in_=kT[0:64, tsl])
                    vt_s = vtmp.tile([128, S], BF16, tag="vtmp",
                                     name=f"vt{strip}")
                    nc.vector.tensor_copy(vt_s[64:128, :], pkv[64:128, :])
                    return vt_s

                state = {}
                if part != "b":
                    pkv = kvpool.tile([128, S], F32, tag=kvtag,
                                      name=f"pkv{strip}")
                    for kc in range(KC):
                        def mk(kc=kc, pkv=pkv):
                            nc.tensor.matmul(
                                pkv, wkv_sb[:, kc, :], xa_strips[strip][kc],
                                start=(kc == 0), stop=(kc == KC - 1))
                            if kc == KC - 1:
                                state["vt_s"] = rope_kv()
                        yield mk

                if part != "a":
                    pq1 = qpool.tile([128, S], F32, tag=qtag,
                                     name=f"pq{strip}_1")
                    for kc in range(KC):
                        def mk(kc=kc):
                            nc.tensor.matmul(
                                pq1, wq_sb[:, kc, 128:256],
                                xa[kc], start=(kc == 0),
                                stop=(kc == KC - 1))
                            if kc == KC - 1:
                                rope_q(1, pq1)
                        yield mk
                if part == "b":
                    return
                for n in range(4):
                    def mk(n=n):
                        pt = mmO.tile([128, 64], BF16, tag="mmO",
                                      name=f"pt{strip}{n}")
                        nc.tensor.transpose(
                            pt, state["vt_s"][64:128, n * 128:(n + 1) * 128],
                            ident[64:128, 64:128])
                        nc.vector.tensor_copy(
                            vaug[:, strip * 4 + n, 0:64], pt)
                    yield mk

            def oproj_filler(strip, ytn, evict_alt=False):
                """Yield closures, each emitting one o_proj matmul; DVE
                eviction into the packed row buffer rides after each group's
                stop, one store DMA per tsub."""
                t0 = strip * S
                for tsub in range(4):
                    trow = t0 + tsub * 128
                    ot = outst.tile([128, D], BF16, tag="out",
                                    name=f"ot{strip}{tsub}")
                    for n in range(4):
                        po = mmO.tile([128, S], F32, tag="mmO",
                                      name=f"po{strip}{tsub}{n}")
                        for c in range(2):
                            def mk(po=po, c=c, tsub=tsub, n=n, trow=trow,
                                   ot=ot):
                                nc.tensor.matmul(
                                    po,
                                    ytn[c][:, tsub * 128:(tsub + 1) * 128],
                                    wo_sb[:, c, n * S:(n + 1) * S],
                                    start=(c == 0), stop=(c == 1),
                                    skip_group_check=True)
                                if c == 1:
                                    if evict_alt and n % 2 == 1:
                                        nc.scalar.copy(
                                            ot[:, n * S:(n + 1) * S], po)
                                    else:
                                        nc.vector.tensor_copy(
                                            ot[:, n * S:(n + 1) * S], po)
                                    if evict_alt:
                                        eng = (nc.gpsimd if n % 2 == 0
                                               else nc.sync)
                                        eng.dma_start(
                                            out=out[trow:trow + 128,
                                                    n * S:(n + 1) * S],
                                            in_=ot[:, n * S:(n + 1) * S])
                                    elif n == 3:
                                        nc.gpsimd.dma_start(
                                            out=out[trow:trow + 128, :],
                                            in_=ot)
                            yield mk

            def run_filler(filler, frac):
                """Emit pending filler ops; frac is how many to emit now."""
                import itertools
                for fn in itertools.islice(filler, frac):
                    fn()

            # strip 0 projection runs dense (nothing to overlap with)
            for fn in proj_filler(0, dense=True):
                fn()

            ytn_strips = {}
            fillers = []  # queue of generators feeding PE gap-filler ops

            for strip in range(NSTRIP):
                t0 = strip * S
                n_sc = (strip + 1) * 4
                ytn = [ytnp.tile([128, S], BF16, tag="ytn",
                                 name=f"ytn{strip}{i}") for i in range(2)]
                ytn_strips[strip] = ytn

                if strip == 0:
                    nc.gpsimd.dma_start(
                        out=wo_sb,
                        in_=woT.rearrange("(c p) n -> p c n", p=128))
                if strip + 1 < NSTRIP:
                    load_xa(strip + 1)
                    fillers.append(proj_filler(strip + 1))
                if strip - 1 >= 0:
                    og = oproj_filler(strip - 1, ytn_strips[strip - 1])
                    if strip == NSTRIP - 2:
                        # give half of this o_proj to the ACT-bound final
                        # strip, where PE slots are free
                        import itertools
                        fillers.append(itertools.islice(og, 16))
                        deferred_oproj = og
                    else:
                        fillers.append(og)
                if strip == NSTRIP - 1:
                    fillers.insert(0, deferred_oproj)

                n_chunks = HPC * n_sc
                pending = 52 if strip + 1 < NSTRIP else 0
                if strip - 1 >= 0:
                    pending += 16 if strip >= NSTRIP - 2 else 32
                if strip == 0:
                    gate = n_chunks // 2
                elif strip + 1 < NSTRIP:
                    gate = n_chunks // 3
                else:
                    gate = 0
                per_chunk = (-(-pending // max(n_chunks - gate, 1))
                             if pending else 0)

                import itertools
                filler_iter = itertools.chain(*fillers)
                fillers = [filler_iter]

                # chunk sequence across heads; even heads (lo=0) first: odd
                # heads need the kT half-dup DMA which lands a bit later
                horder = (1, 3, 0, 2) if strip == NSTRIP - 1 else (0, 2, 1, 3)
                seq = [(h, j) for h in horder for j in range(n_sc)]
                pq1_iter = None

                def emit_scores(h, j):
                    """Scores matmul + diag mask + exp; returns state the
                    deferred attn@v needs."""
                    hp, lo = h // 2, (h % 2) * 64
                    o = max(j * 128 - t0, 0)
                    if strip == NSTRIP - 1 and (h + j) % 2 == 1:
                        # final strip has no projection fillers, so the mmP
                        # ring is idle -- alternating pools doubles the
                        # effective scores-ring depth and unchains the
                        # chunk period from the exp latency
                        ps_sc = mmP.tile([128, S], F32, tag="mmP",
                                         name=f"s{strip}{h}{j}")
                    else:
                        ps_sc = mmS.tile([128, S], F32, tag="mmS",
                                         name=f"s{strip}{h}{j}")
                    diag = j * 128 - t0 >= 0
                    nc.tensor.matmul(
                        ps_sc[:, o:S],
                        kT[lo:lo + 64, j * 128:(j + 1) * 128],
                        qT[hp][lo:lo + 64, t0 + o:t0 + S],
                        start=True, stop=True, skip_group_check=True)
                    ex = expp.tile([128, S], BF16, tag="exp",
                                   name=f"e{strip}{h}{j}")
                    nc.scalar.activation(
                        ex[:, o:S], ps_sc[:, o:S],
                        mybir.ActivationFunctionType.Exp,
                        scale=1.0 / math.sqrt(HD))
                    if diag:
                        nc.gpsimd.affine_select(
                            out=ex[:, o:o + 128], in_=ex[:, o:o + 128],
                            pattern=[[1, 128]], base=0,
                            channel_multiplier=-1,
                            compare_op=mybir.AluOpType.is_ge, fill=0.0)
                    if debug and strip == 0 and h == 0:
                        nc.sync.dma_start(
                            out=dbg["d_exp"][:, j * S:(j + 1) * S], in_=ex)
                    return (h, j, o, ex)

                def emit_attnv(st, yt_ps):
                    h, j, o, ex = st
                    nc.tensor.matmul(
                        yt_ps[0:65, o:S], vaug[:, j, :], ex[:, o:S],
                        start=(j == 0), stop=(j == n_sc - 1),
                        skip_group_check=True)

                def emit_normalize_a(h, yt_ps):
                    """Stage A (right after the closing attn@v): reciprocal
                    of the denom row on the DVE, cast to bf16 for the PE
                    broadcast matmul."""
                    dn = dnp.tile([128, S], F32, tag="dnr",
                                  bufs=2, name=f"dnr{strip}{h}")
                    # the custom op mis-executes on 1-row slices; run it on
                    # the full tile (rows != 64 are unused garbage)
                    nc.vector.reciprocal_approx_fast(out=dn, in_=yt_ps)
                    dnb = dnp.tile([128, S], BF16, tag="dnb",
                                   bufs=2, name=f"dnb{strip}{h}")
                    nc.vector.tensor_copy(dnb[64:65, :], dn[64:65, :])
                    return dnb

                def dump_norm(h, yt_ps, dn, bc_t):
                    if debug and strip == 0 and h == 0:
                        yd = dnp.tile([128, S], F32, tag="dn", name="yd")
                        nc.vector.tensor_copy(yd, yt_ps)
                        nc.sync.dma_start(out=dbg["d_yt"], in_=yd)
                        dnf = dnp.tile([128, S], F32, tag="dn", name="dnf")
                        nc.vector.tensor_copy(dnf[64:65, :], dn[64:65, :])
                        nc.sync.dma_start(out=dbg["d_dn"], in_=dnf)
                        nc.sync.dma_start(out=dbg["d_bc"], in_=bc_t)

                def emit_normalize_b(h, yt_ps, dn):
                    """Stage B (two chunks later): PE outer-product
                    broadcast of the 1/denom row across partitions, then the
                    DVE normalize mul; odd heads land on partitions 64-127
                    of ytn via a gpsimd copy."""
                    hp, odd = h // 2, h % 2
                    bc_ps = mmO.tile([64, S], F32, tag="mmO",
                                      name=f"bp{strip}{h}")
                    nc.tensor.matmul(
                        bc_ps, ones_b[64:65, :], dn[64:65, :],
                        start=True, stop=True, skip_group_check=True)
                    bc_t = dnp.tile([128, S], F32, tag="dn",
                                    name=f"bc{strip}{h}")
                    nc.vector.tensor_copy(bc_t[0:64, :], bc_ps)
                    dump_norm(h, yt_ps, dn, bc_t)
                    if not odd:
                        nc.vector.tensor_mul(
                            ytn[hp][0:64, :], yt_ps[0:64, :], bc_t[0:64, :])
                    else:
                        ntmp = dnp.tile([128, S], BF16, tag="ntmp", bufs=2,
                                        name=f"nt{strip}{h}")
                        nc.vector.tensor_mul(
                            ntmp[0:64, :], yt_ps[0:64, :], bc_t[0:64, :])
                        nc.gpsimd.dma_start(
                            out=ytn[hp][64:128, :], in_=ntmp[0:64, :])

                LA = 2  # scores run LA chunks ahead of attn@v
                pending_ops = []  # (due_idx, closure)
                inflight = []     # [(st, yt)] scores awaiting attn@v
                yt_cur = None

                def retire(idx):
                    st, cyt = inflight.pop(0)
                    emit_attnv(st, cyt)
                    if st[1] == n_sc - 1:  # closing chunk of a head
                        ch = st[0]
                        dn = emit_normalize_a(ch, cyt)
                        pending_ops.append(
                            (idx + 2,
                             lambda ch=ch, cyt=cyt, dn=dn:
                             emit_normalize_b(ch, cyt, dn)))

                for idx, (h, j) in enumerate(seq):
                    for due, fn in [p for p in pending_ops if p[0] <= idx]:
                        fn()
                    pending_ops = [p for p in pending_ops if p[0] > idx]
                    if j == 0:
                        yt_cur = ytps.tile([128, S], F32, tag="yt",
                                           name=f"yt{strip}{h}")
                    st = emit_scores(h, j)
                    inflight.append((st, yt_cur))
                    if len(inflight) > LA:
                        retire(idx)
                        if pq1_iter is not None and idx >= 1:
                            run_filler(pq1_iter, 8)
                        if idx >= gate:
                            run_filler(filler_iter, per_chunk)

                # close out the strip
                idx = len(seq)
                while inflight:
                    retire(idx)
                    idx += 1
                for due, fn in sorted(pending_ops):
                    fn()

                if debug and strip == 0:
                    nc.sync.dma_start(out=dbg["d_qT0"], in_=qT[0])
                    nc.sync.dma_start(out=dbg["d_qT1"], in_=qT[1])
                    nc.sync.dma_start(out=dbg["d_kT"], in_=kT)
                    nc.sync.dma_start(
                        out=dbg["d_vaug"],
                        in_=vaug.rearrange("p a b -> p (a b)"))
                    nc.sync.dma_start(out=dbg["d_ytn0"], in_=ytn[0])
                    nc.sync.dma_start(out=dbg["d_ytn1"], in_=ytn[1])

                # drain any leftover filler before the next strip
                for fn in filler_iter:
                    fn()
                fillers = []

            # last strip's o_proj runs dense at the tail
            for fn in oproj_filler(NSTRIP - 1, ytn_strips[NSTRIP - 1],
                                   evict_alt=True):
                fn()

    nc.compile()
    return nc


_NC_CACHE = None


def _get_nc():
    global _NC_CACHE
    if _NC_CACHE is None:
        _NC_CACHE = _build_kernel()
    return _NC_CACHE


def _prep_inputs(x, wq, wk, wv, wo):
    """Host-side shard + layout prep. Returns per-core input maps."""
    import ml_dtypes
    bf16 = ml_dtypes.bfloat16

    x = np.asarray(x, dtype=np.float32).reshape(T, D)
    wq = np.asarray(wq, dtype=np.float32)
    wk = np.asarray(wk, dtype=np.float32)
    wv = np.asarray(wv, dtype=np.float32)
    wo = np.asarray(wo, dtype=np.float32)

    xT_b = np.ascontiguousarray(x.T).astype(bf16)

    # head-dim permutation for rope: [even pair comps | odd pair comps]
    perm = np.concatenate([np.arange(0, HD, 2), np.arange(1, HD, 2)])

    # rope tables in the [d, t] layout
    theta = 1.0 / ROPE_BASE ** (np.arange(0, HD, 2, dtype=np.float64) / HD)
    ang = np.arange(T, dtype=np.float64)[None, :] * theta[:, None]  # [32, T]
    cos_blk = np.cos(ang).astype(np.float32)
    sin_blk = np.sin(ang).astype(np.float32)
    cosT = np.tile(np.concatenate([cos_blk, cos_blk], 0), (2, 1))
    sinT = np.tile(np.concatenate([sin_blk, -sin_blk], 0), (2, 1))
    cosT = np.ascontiguousarray(cosT)
    sinT = np.ascontiguousarray(sinT)

    in_maps = []
    for c in range(NCORES):
        wq_c = wq[c * QC:(c + 1) * QC].reshape(HPC, HD, D)[:, perm, :]
        wq_c = wq_c.reshape(QC, D)
        wk_c = wk[c * HD:(c + 1) * HD][perm, :]
        wv_c = wv[c * HD:(c + 1) * HD]
        wkv_c = np.concatenate([wk_c, wv_c], axis=0)          # [128, D]
        wo_c = wo[:, c * QC:(c + 1) * QC]                      # [D, QC]
        in_maps.append({
            "xT": xT_b,
            "wqT": np.ascontiguousarray(wq_c.T).astype(bf16),
            "wkvT": np.ascontiguousarray(wkv_c.T).astype(bf16),
            "woT": np.ascontiguousarray(wo_c.T).astype(bf16),
            "cosT": cosT.astype(bf16),
            "sinT": sinT.astype(bf16),
        })
    return in_maps


def kernel(x, wq, wk, wv, wo):
    from concourse.bass_utils import run_bass_kernel_spmd

    nc = _get_nc()
    in_maps = _prep_inputs(x, wq, wk, wv, wo)
    res = run_bass_kernel_spmd(nc, in_maps, core_ids=list(range(NCORES)))
    acc = np.zeros((T, D), dtype=np.float64)
    for c in range(NCORES):
        acc += res.results[c]["out"].astype(np.float64)
    return acc.astype(np.float32).reshape(1, T, D)


# revision 35
# speedup vs baseline: 1.0331x; 1.0331x over previous
"""Causal self-attention (GQA + RoPE) for TRN2, sharded over 8 NeuronCores.

Sharding: tensor-parallel over heads. Each core owns 4 query heads and 1 KV
head (H=32, HKV=8 -> group size 4). Column-parallel q/k/v projections,
row-parallel o_proj; the final all-reduce over the 8 partial [T, D] outputs
happens on the host after the gather.

Performance design (377.8us f32r baseline -> 214.4us):
  - All matmul operands are bf16 (PSUM accumulation stays fp32): halves
    every DMA and LDWEIGHTS. PSUM f32 throughout; rel err 4.4e-3 vs 2e-2
    tolerance.
  - Software-pipelined emission with lookahead 2: scores for chunks j+1,
    j+2 are issued to the in-order PE queue BEFORE attn@v of chunk j, so
    the PE never head-of-line blocks on the ACT exp of the current chunk
    (the baseline idled the PE ~50% through every attention phase, which
    also dropped the PE p-state clock from 2.4 to 1.2 GHz).
  - Projection of strip s+1 and o_proj of strip s-1 ride as PE gap fillers
    inside strip s's attention, gated to the last 2/3 of the strip so they
    never wait on in-flight input DMAs; half of the second-to-last o_proj
    is deferred into the final strip, whose attention is otherwise
    ACT-bound.
  - PSUM pools are split by lifetime class (scores / projection
    accumulators / o_proj+transpose+broadcast / attn accumulators =
    2+2+2+2 banks) so fast-churning score tiles never recycle behind a
    16-matmul projection group. In the final strip (no projection
    fillers) the scores alternate between the two rings, doubling the
    effective ring depth and unchaining the chunk period from the exp
    latency.
  - Softmax reciprocal runs on the DVE (reciprocal_approx_fast custom op,
    full-tile form -- the 1-row slice form mis-executes) so ACT keeps its
    Exp table loaded for the whole kernel (the baseline burned 41us in
    ACT_TABLE_LOAD ping-pong); the 1/denom row is broadcast across
    partitions by a K=1 PE outer product.
  - o_proj eviction copies run on the DVE (final strip: alternating
    DVE/ACT), writing bf16 into a packed [128, D] row buffer; stores are
    one DMA per (strip, tsub), split across two queues at the tail.
  - Startup loads are split across the sync/scalar/gpsimd queues (SWDGE
    descriptor generation is 3-7us per big rearranged DMA and serializes
    per queue), ordered so the first projection chunk lands in ~5us; x
    strips load as one 3D-rearranged DMA per strip, double buffered.
Layout (all on-chip tensors keep the contraction dim on partitions):
  - x is uploaded pre-transposed as xT [D, T]; projections produce qT/kT/vT
    [d, t] directly in PSUM.
  - RoPE uses a host-side permutation of the head dim into [even-pairs |
    odd-pairs] halves so the pair rotation becomes: out = q*cos +
    swap64(q*sin'), where swap64 swaps 32-row halves within each 64-row
    head block (SBUF->SBUF DMAs; q on sync queue, k on gpsimd) and sin'
    carries the sign pattern [+sin | -sin].
  - Scores are computed transposed (scoresT [s, t]) so the softmax
    denominator comes out of the attn@v matmul for free via the ones
    column in the v stationary operand; causal masking of diagonal blocks
    is a post-exp gpsimd affine_select.
"""

import math

import numpy as np

import concourse.bass as bass
import concourse.mybir as mybir
import concourse.tile as tile
from concourse import bacc
from concourse.masks import make_identity

D = 2048
H = 32
HKV = 8
HD = 64
T = 2048
NCORES = 8
HPC = H // NCORES        # 4 query heads per core
QC = HPC * HD            # 256 q dims per core
ROPE_BASE = 10000.0
S = 512                  # t-strip / moving-operand width
NSTRIP = T // S          # 4
KC = D // 128            # 16 contraction chunks

F32 = mybir.dt.float32
BF16 = mybir.dt.bfloat16


def _build_kernel(debug=False):
    nc = bacc.Bacc("TRN2", target_bir_lowering=False, debug=False,
                   num_devices=NCORES)

    xT = nc.dram_tensor("xT", [D, T], BF16, kind="ExternalInput").ap()
    wqT = nc.dram_tensor("wqT", [D, QC], BF16, kind="ExternalInput").ap()
    wkvT = nc.dram_tensor("wkvT", [D, 128], BF16, kind="ExternalInput").ap()
    woT = nc.dram_tensor("woT", [QC, D], BF16, kind="ExternalInput").ap()
    cosT = nc.dram_tensor("cosT", [128, T], BF16,
                          kind="ExternalInput").ap()
    sinT = nc.dram_tensor("sinT", [128, T], BF16,
                          kind="ExternalInput").ap()
    out = nc.dram_tensor("out", [T, D], BF16, kind="ExternalOutput").ap()
    dbg = {}
    if debug:
        for nm, shp, dt in [("d_qT0", [128, T], BF16), ("d_qT1", [128, T], BF16),
                            ("d_kT", [128, T], BF16),
                            ("d_vaug", [128, 16 * 65], BF16),
                            ("d_exp", [128, 4 * S], BF16),
                            ("d_yt", [128, S], F32), ("d_dn", [128, S], F32),
                            ("d_bc", [128, S], F32),
                            ("d_ytn0", [128, S], BF16),
                            ("d_ytn1", [128, S], BF16)]:
            dbg[nm] = nc.dram_tensor(nm, shp, dt, kind="ExternalOutput").ap()

    with tile.TileContext(nc) as tc:
        with (
            tc.tile_pool(name="consts", bufs=1) as consts,
            tc.tile_pool(name="persist", bufs=1) as persist,
            tc.tile_pool(name="xa", bufs=3) as xap,
            tc.tile_pool(name="rtmp", bufs=6) as rtmp,
            tc.tile_pool(name="swp", bufs=4) as swp,
            tc.tile_pool(name="vtmp", bufs=2) as vtmp,
            tc.tile_pool(name="expp", bufs=6) as expp,
            tc.tile_pool(name="ytn", bufs=6) as ytnp,
            tc.tile_pool(name="outst", bufs=3) as outst,
            tc.tile_pool(name="dn", bufs=4) as dnp,
            tc.tile_pool(name="mmS", bufs=2, space="PSUM") as mmS,
            tc.tile_pool(name="mmP", bufs=2, space="PSUM") as mmP,
            tc.tile_pool(name="mmO", bufs=2, space="PSUM") as mmO,
            tc.tile_pool(name="ytps", bufs=2, space="PSUM") as ytps,
        ):
            # ---- constants; DMA issue order interleaved per k-chunk so the
            # first projection matmul only waits on chunk 0 of wq/x ----
            wq_sb = consts.tile([128, KC, QC], BF16)
            wkv_sb = consts.tile([128, KC, 128], BF16)
            xa_strips = {}

            def load_xa(strip):
                t0 = strip * S
                xt = xap.tile([128, KC, S], BF16, tag="xa",
                              name=f"xa{strip}")
                nc.sync.dma_start(
                    out=xt,
                    in_=xT.rearrange("(c p) t -> p c t", p=128)[:, :,
                                                               t0:t0 + S])
                xa_strips[strip] = [xt[:, kc, :] for kc in range(KC)]

            wqT_r = wqT.rearrange("(c p) q -> p c q", p=128)
            xT_r = xT.rearrange("(c p) t -> p c t", p=128)
            # startup loads spread across 4 queues so SWDGE descriptor
            # generation (3-7us per big rearranged DMA) runs in parallel
            # staircase the entry loads in consumption order across three
            # queues so projection chunks land incrementally instead of in
            # two big all-at-once waves
            xt0 = xap.tile([128, KC, S], BF16, tag="xa", name="xa0")
            nc.sync.dma_start(out=xt0[:, 0:4, :], in_=xT_r[:, 0:4, 0:S])
            nc.scalar.dma_start(out=wq_sb[:, 0:4, :], in_=wqT_r[:, 0:4, :])
            nc.gpsimd.dma_start(
                out=wkv_sb, in_=wkvT.rearrange("(c p) q -> p c q", p=128))
            nc.sync.dma_start(out=xt0[:, 4:8, :], in_=xT_r[:, 4:8, 0:S])
            nc.scalar.dma_start(out=wq_sb[:, 4:10, :], in_=wqT_r[:, 4:10, :])
            nc.sync.dma_start(out=xt0[:, 8:12, :], in_=xT_r[:, 8:12, 0:S])
            nc.gpsimd.dma_start(out=xt0[:, 12:KC, :],
                                in_=xT_r[:, 12:KC, 0:S])
            nc.scalar.dma_start(out=wq_sb[:, 10:KC, :],
                                in_=wqT_r[:, 10:KC, :])
            xa_strips[0] = [xt0[:, kc, :] for kc in range(KC)]
            cs_c = consts.tile([128, T], BF16)
            cs_s = consts.tile([128, T], BF16)
            nc.scalar.dma_start(out=cs_c, in_=cosT)
            nc.sync.dma_start(out=cs_s, in_=sinT)
            load_xa(1)
            wo_sb = consts.tile([128, 2, D], BF16)
            ident_f = consts.tile([128, 128], F32)
            make_identity(nc, ident_f)
            ident = consts.tile([128, 128], BF16)
            nc.vector.tensor_copy(ident, ident_f)
            # PE warmup: keep the array continuously busy while the first
            # input DMAs land so the p-state ramp is done by the first
            # projection matmul (fp32 on purpose: 4 cyc/row keeps the array
            # busy longer per instruction)
            warm_ps = mmO.tile([128, 512], F32, tag="mmO", name="warm")
            junk = consts.tile([128, 512], F32)
            nc.vector.memset(junk, 1.0)
            for w in range(2):
                nc.tensor.matmul(
                    warm_ps, ident_f, junk,
                    start=True, stop=True, skip_group_check=True)
            ones_b = consts.tile([128, 64], BF16)
            nc.vector.memset(ones_b, 1.0)

            # persistent activations
            qT = [persist.tile([128, T], BF16, tag=f"qT{i}", name=f"qT{i}")
                  for i in range(2)]
            # k duplicated on both partition halves so each q head can use
            # a stationary slice whose base partition matches its rhs base
            kT = persist.tile([128, T], BF16)
            # vaug columns: [v(64) | ones]; the ones column makes the
            # softmax denominator fall out of the attn@v matmul (row 64)
            vaug = persist.tile([128, 4 * NSTRIP, 65], BF16)
            ones_col = consts.tile([128, 4 * NSTRIP, 1], F32)
            nc.vector.memset(ones_col, 1.0)
            nc.vector.tensor_copy(vaug[:, :, 64:65], ones_col)

            def proj_filler(strip, dense=False, part=None):
                """Yield closures, each emitting one PE op of this strip's
                q/kv projection; rope/evict DVE work rides along after the
                last matmul of each accumulation group. The dense (pre-loop)
                call borrows the then-idle mmS ring for the q tiles so the
                three groups are not serialized by the 1-slot mmP ring."""
                qpool, qtag = (mmP, "mmP")
                kvpool, kvtag = (mmS, "mmS") if dense else (mmP, "mmP")
                t0 = strip * S
                tsl = slice(t0, t0 + S)
                xa = xa_strips[strip]

                def rope_q(hp, pq):
                    qc = rtmp.tile([128, S], F32, tag="rtmp",
                                   name=f"qc{strip}{hp}")
                    qs = rtmp.tile([128, S], F32, tag="rtmp",
                                   name=f"qs{strip}{hp}")
                    nc.vector.tensor_mul(qs, pq, cs_s[:, tsl])
                    sw = swp.tile([128, S], F32, tag="swp",
                                  name=f"sw{strip}{hp}")
                    for b in range(2):
                        nc.sync.dma_start(
                            out=sw[b * 64:b * 64 + 32, :],
                            in_=qs[b * 64 + 32:b * 64 + 64, :])
                        nc.sync.dma_start(
                            out=sw[b * 64 + 32:b * 64 + 64, :],
                            in_=qs[b * 64:b * 64 + 32, :])
                    nc.vector.tensor_mul(qc, pq, cs_c[:, tsl])
                    nc.vector.tensor_add(qT[hp][:, tsl], qc, sw)

                if part != "b":
                    pq0 = qpool.tile([128, S], F32, tag=qtag,
                                     name=f"pq{strip}_0")
                    for kc in range(KC):
                        def mk(kc=kc):
                            nc.tensor.matmul(
                                pq0, wq_sb[:, kc, 0:128],
                                xa[kc], start=(kc == 0),
                                stop=(kc == KC - 1))
                            if kc == KC - 1:
                                rope_q(0, pq0)
                        yield mk

                def rope_kv():
                    kc_t = rtmp.tile([128, S], F32, tag="rtmp",
                                     name=f"kc{strip}")
                    ks_t = rtmp.tile([128, S], F32, tag="rtmp",
                                     name=f"ks{strip}")
                    nc.vector.tensor_mul(
                        ks_t[0:64, :], pkv[0:64, :], cs_s[0:64, tsl])
                    swk = swp.tile([128, S], F32, tag="swp",
                                   name=f"swk{strip}")
                    nc.gpsimd.dma_start(out=swk[0:32, :], in_=ks_t[32:64, :])
                    nc.gpsimd.dma_start(out=swk[32:64, :], in_=ks_t[0:32, :])
                    nc.vector.tensor_mul(
                        kc_t[0:64, :], pkv[0:64, :], cs_c[0:64, tsl])
                    nc.vector.tensor_add(
                        kT[0:64, tsl], kc_t[0:64, :], swk[0:64, :])
                    nc.gpsimd.dma_start(out=kT[64:128, tsl], in_=kT[0:64, tsl])
                    vt_s = vtmp.tile([128, S], BF16, tag="vtmp",
                                     name=f"vt{strip}")
                    nc.vector.tensor_copy(vt_s[64:128, :], pkv[64:128, :])
                    return vt_s

                state = {}
                if part != "b":
                    pkv = kvpool.tile([128, S], F32, tag=kvtag,
                                      name=f"pkv{strip}")
                    for kc in range(KC):
                        def mk(kc=kc, pkv=pkv):
                            nc.tensor.matmul(
                                pkv, wkv_sb[:, kc, :], xa_strips[strip][kc],
                                start=(kc == 0), stop=(kc == KC - 1))
                            if kc == KC - 1:
                                state["vt_s"] = rope_kv()
                        yield mk

                if part != "a":
                    pq1 = qpool.tile([128, S], F32, tag=qtag,
                                     name=f"pq{strip}_1")
                    for kc in range(KC):
                        def mk(kc=kc):
                            nc.tensor.matmul(
                                pq1, wq_sb[:, kc, 128:256],
                                xa[kc], start=(kc == 0),
                                stop=(kc == KC - 1))
                            if kc == KC - 1:
                                rope_q(1, pq1)
                        yield mk
                if part == "b":
                    return
                for n in range(4):
                    def mk(n=n):
                        pt = mmO.tile([128, 64], BF16, tag="mmO",
                                      name=f"pt{strip}{n}")
                        nc.tensor.transpose(
                            pt, state["vt_s"][64:128, n * 128:(n + 1) * 128],
                            ident[64:128, 64:128])
                        nc.vector.tensor_copy(
                            vaug[:, strip * 4 + n, 0:64], pt)
                    yield mk

            def oproj_filler(strip, ytn, evict_alt=False):
                """Yield closures, each emitting one o_proj matmul; DVE
                eviction into the packed row buffer rides after each group's
                stop, one store DMA per tsub."""
                t0 = strip * S
                for tsub in range(4):
                    trow = t0 + tsub * 128
                    ot = outst.tile([128, D], BF16, tag="out",
                                    name=f"ot{strip}{tsub}")
                    for n in range(4):
                        po = mmO.tile([128, S], F32, tag="mmO",
                                      name=f"po{strip}{tsub}{n}")
                        for c in range(2):
                            def mk(po=po, c=c, tsub=tsub, n=n, trow=trow,
                                   ot=ot):
                                nc.tensor.matmul(
                                    po,
                                    ytn[c][:, tsub * 128:(tsub + 1) * 128],
                                    wo_sb[:, c, n * S:(n + 1) * S],
                                    start=(c == 0), stop=(c == 1),
                                    skip_group_check=True)
                                if c == 1:
                                    if evict_alt and n % 2 == 1:
                                        nc.scalar.copy(
                                            ot[:, n * S:(n + 1) * S], po)
                                    else:
                                        nc.vector.tensor_copy(
                                            ot[:, n * S:(n + 1) * S], po)
                                    if evict_alt:
                                        eng = (nc.gpsimd if n % 2 == 0
                                               else nc.sync)
                                        eng.dma_start(
                                            out=out[trow:trow + 128,
                                                    n * S:(n + 1) * S],
                                            in_=ot[:, n * S:(n + 1) * S])
                                    elif n == 3:
                                        nc.gpsimd.dma_start(
                                            out=out[trow:trow + 128, :],
                                            in_=ot)
                            yield mk

            def run_filler(filler, frac):
                """Emit pending filler ops; frac is how many to emit now."""
                import itertools
                for fn in itertools.islice(filler, frac):
                    fn()

            # strip 0 projection runs dense (nothing to overlap with)
            for fn in proj_filler(0, dense=True):
                fn()

            ytn_strips = {}
            fillers = []  # queue of generators feeding PE gap-filler ops

            for strip in range(NSTRIP):
                t0 = strip * S
                n_sc = (strip + 1) * 4
                ytn = [ytnp.tile([128, S], BF16, tag="ytn",
                                 name=f"ytn{strip}{i}") for i in range(2)]
                ytn_strips[strip] = ytn

                if strip == 0:
                    nc.gpsimd.dma_start(
                        out=wo_sb,
                        in_=woT.rearrange("(c p) n -> p c n", p=128))
                if strip + 1 < NSTRIP:
                    if strip + 1 not in xa_strips:
                        load_xa(strip + 1)
                    fillers.append(proj_filler(strip + 1))
                if strip - 1 >= 0:
                    og = oproj_filler(strip - 1, ytn_strips[strip - 1])
                    if strip == NSTRIP - 2:
                        # give half of this o_proj to the ACT-bound final
                        # strip, where PE slots are free
                        import itertools
                        fillers.append(itertools.islice(og, 16))
                        deferred_oproj = og
                    else:
                        fillers.append(og)
                if strip == NSTRIP - 1:
                    fillers.insert(0, deferred_oproj)

                n_chunks = HPC * n_sc
                pending = 52 if strip + 1 < NSTRIP else 0
                if strip - 1 >= 0:
                    pending += 16 if strip >= NSTRIP - 2 else 32
                if strip == 0:
                    gate = n_chunks // 2
                elif strip + 1 < NSTRIP:
                    gate = n_chunks // 3
                else:
                    gate = 0
                per_chunk = (-(-pending // max(n_chunks - gate, 1))
                             if pending else 0)

                import itertools
                filler_iter = itertools.chain(*fillers)
                fillers = [filler_iter]

                # chunk sequence across heads; even heads (lo=0) first: odd
                # heads need the kT half-dup DMA which lands a bit later
                horder = (1, 3, 0, 2) if strip == NSTRIP - 1 else (0, 2, 1, 3)
                seq = [(h, j) for h in horder for j in range(n_sc)]
                pq1_iter = None

                def emit_scores(h, j):
                    """Scores matmul + diag mask + exp; returns state the
                    deferred attn@v needs."""
                    hp, lo = h // 2, (h % 2) * 64
                    o = max(j * 128 - t0, 0)
                    if strip == NSTRIP - 1 and (h + j) % 2 == 1:
                        # final strip has no projection fillers, so the mmP
                        # ring is idle -- alternating pools doubles the
                        # effective scores-ring depth and unchains the
                        # chunk period from the exp latency
                        ps_sc = mmP.tile([128, S], F32, tag="mmP",
                                         name=f"s{strip}{h}{j}")
                    else:
                        ps_sc = mmS.tile([128, S], F32, tag="mmS",
                                         name=f"s{strip}{h}{j}")
                    diag = j * 128 - t0 >= 0
                    nc.tensor.matmul(
                        ps_sc[:, o:S],
                        kT[lo:lo + 64, j * 128:(j + 1) * 128],
                        qT[hp][lo:lo + 64, t0 + o:t0 + S],
                        start=True, stop=True, skip_group_check=True)
                    ex = expp.tile([128, S], BF16, tag="exp",
                                   name=f"e{strip}{h}{j}")
                    nc.scalar.activation(
                        ex[:, o:S], ps_sc[:, o:S],
                        mybir.ActivationFunctionType.Exp,
                        scale=1.0 / math.sqrt(HD))
                    if diag:
                        nc.gpsimd.affine_select(
                            out=ex[:, o:o + 128], in_=ex[:, o:o + 128],
                            pattern=[[1, 128]], base=0,
                            channel_multiplier=-1,
                            compare_op=mybir.AluOpType.is_ge, fill=0.0)
                    if debug and strip == 0 and h == 0:
                        nc.sync.dma_start(
                            out=dbg["d_exp"][:, j * S:(j + 1) * S], in_=ex)
                    return (h, j, o, ex)

                def emit_attnv(st, yt_ps):
                    h, j, o, ex = st
                    nc.tensor.matmul(
                        yt_ps[0:65, o:S], vaug[:, j, :], ex[:, o:S],
                        start=(j == 0), stop=(j == n_sc - 1),
                        skip_group_check=True)

                def emit_normalize_a(h, yt_ps):
                    """Stage A (right after the closing attn@v): reciprocal
                    of the denom row on the DVE, cast to bf16 for the PE
                    broadcast matmul."""
                    dn = dnp.tile([128, S], F32, tag="dnr",
                                  bufs=2, name=f"dnr{strip}{h}")
                    # the custom op mis-executes on 1-row slices; run it on
                    # the full tile (rows != 64 are unused garbage)
                    nc.vector.reciprocal_approx_fast(out=dn, in_=yt_ps)
                    dnb = dnp.tile([128, S], BF16, tag="dnb",
                                   bufs=2, name=f"dnb{strip}{h}")
                    nc.vector.tensor_copy(dnb[64:65, :], dn[64:65, :])
                    return dnb

                def dump_norm(h, yt_ps, dn, bc_t):
                    if debug and strip == 0 and h == 0:
                        yd = dnp.tile([128, S], F32, tag="dn", name="yd")
                        nc.vector.tensor_copy(yd, yt_ps)
                        nc.sync.dma_start(out=dbg["d_yt"], in_=yd)
                        dnf = dnp.tile([128, S], F32, tag="dn", name="dnf")
                        nc.vector.tensor_copy(dnf[64:65, :], dn[64:65, :])
                        nc.sync.dma_start(out=dbg["d_dn"], in_=dnf)
                        nc.sync.dma_start(out=dbg["d_bc"], in_=bc_t)

                def emit_normalize_b(h, yt_ps, dn):
                    """Stage B (two chunks later): PE outer-product
                    broadcast of the 1/denom row across partitions, then the
                    DVE normalize mul; odd heads land on partitions 64-127
                    of ytn via a gpsimd copy."""
                    hp, odd = h // 2, h % 2
                    bc_ps = mmO.tile([64, S], F32, tag="mmO",
                                      name=f"bp{strip}{h}")
                    nc.tensor.matmul(
                        bc_ps, ones_b[64:65, :], dn[64:65, :],
                        start=True, stop=True, skip_group_check=True)
                    bc_t = dnp.tile([128, S], F32, tag="dn",
                                    name=f"bc{strip}{h}")
                    nc.vector.tensor_copy(bc_t[0:64, :], bc_ps)
                    dump_norm(h, yt_ps, dn, bc_t)
                    if not odd:
                        nc.vector.tensor_mul(
                            ytn[hp][0:64, :], yt_ps[0:64, :], bc_t[0:64, :])
                    else:
                        ntmp = dnp.tile([128, S], BF16, tag="ntmp", bufs=2,
                                        name=f"nt{strip}{h}")
                        nc.vector.tensor_mul(
                            ntmp[0:64, :], yt_ps[0:64, :], bc_t[0:64, :])
                        nc.gpsimd.dma_start(
                            out=ytn[hp][64:128, :], in_=ntmp[0:64, :])

                LA = 2  # scores run LA chunks ahead of attn@v
                pending_ops = []  # (due_idx, closure)
                inflight = []     # [(st, yt)] scores awaiting attn@v
                yt_cur = None

                def retire(idx):
                    st, cyt = inflight.pop(0)
                    emit_attnv(st, cyt)
                    if st[1] == n_sc - 1:  # closing chunk of a head
                        ch = st[0]
                        dn = emit_normalize_a(ch, cyt)
                        pending_ops.append(
                            (idx + 2,
                             lambda ch=ch, cyt=cyt, dn=dn:
                             emit_normalize_b(ch, cyt, dn)))

                for idx, (h, j) in enumerate(seq):
                    for due, fn in [p for p in pending_ops if p[0] <= idx]:
                        fn()
                    pending_ops = [p for p in pending_ops if p[0] > idx]
                    if j == 0:
                        yt_cur = ytps.tile([128, S], F32, tag="yt",
                                           name=f"yt{strip}{h}")
                    st = emit_scores(h, j)
                    inflight.append((st, yt_cur))
                    if len(inflight) > LA:
                        retire(idx)
                        if pq1_iter is not None and idx >= 1:
                            run_filler(pq1_iter, 8)
                        if idx >= gate:
                            run_filler(filler_iter, per_chunk)

                # close out the strip
                idx = len(seq)
                while inflight:
                    retire(idx)
                    idx += 1
                for due, fn in sorted(pending_ops):
                    fn()

                if debug and strip == 0:
                    nc.sync.dma_start(out=dbg["d_qT0"], in_=qT[0])
                    nc.sync.dma_start(out=dbg["d_qT1"], in_=qT[1])
                    nc.sync.dma_start(out=dbg["d_kT"], in_=kT)
                    nc.sync.dma_start(
                        out=dbg["d_vaug"],
                        in_=vaug.rearrange("p a b -> p (a b)"))
                    nc.sync.dma_start(out=dbg["d_ytn0"], in_=ytn[0])
                    nc.sync.dma_start(out=dbg["d_ytn1"], in_=ytn[1])

                # drain any leftover filler before the next strip
                for fn in filler_iter:
                    fn()
                fillers = []

            # last strip's o_proj runs dense at the tail
            for fn in oproj_filler(NSTRIP - 1, ytn_strips[NSTRIP - 1],
                                   evict_alt=True):
                fn()

    nc.compile()
    return nc


_NC_CACHE = None


def _get_nc():
    global _NC_CACHE
    if _NC_CACHE is None:
        _NC_CACHE = _build_kernel()
    return _NC_CACHE


def _prep_inputs(x, wq, wk, wv, wo):
    """Host-side shard + layout prep. Returns per-core input maps."""
    import ml_dtypes
    bf16 = ml_dtypes.bfloat16

    x = np.asarray(x, dtype=np.float32).reshape(T, D)
    wq = np.asarray(wq, dtype=np.float32)
    wk = np.asarray(wk, dtype=np.float32)
    wv = np.asarray(wv, dtype=np.float32)
    wo = np.asarray(wo, dtype=np.float32)

    xT_b = np.ascontiguousarray(x.T).astype(bf16)

    # head-dim permutation for rope: [even pair comps | odd pair comps]
    perm = np.concatenate([np.arange(0, HD, 2), np.arange(1, HD, 2)])

    # rope tables in the [d, t] layout
    theta = 1.0 / ROPE_BASE ** (np.arange(0, HD, 2, dtype=np.float64) / HD)
    ang = np.arange(T, dtype=np.float64)[None, :] * theta[:, None]  # [32, T]
    cos_blk = np.cos(ang).astype(np.float32)
    sin_blk = np.sin(ang).astype(np.float32)
    cosT = np.tile(np.concatenate([cos_blk, cos_blk], 0), (2, 1))
    sinT = np.tile(np.concatenate([sin_blk, -sin_blk], 0), (2, 1))
    cosT = np.ascontiguousarray(cosT)
    sinT = np.ascontiguousarray(sinT)

    in_maps = []
    for c in range(NCORES):
        wq_c = wq[c * QC:(c + 1) * QC].reshape(HPC, HD, D)[:, perm, :]
        wq_c = wq_c.reshape(QC, D)
        wk_c = wk[c * HD:(c + 1) * HD][perm, :]
        wv_c = wv[c * HD:(c + 1) * HD]
        wkv_c = np.concatenate([wk_c, wv_c], axis=0)          # [128, D]
        wo_c = wo[:, c * QC:(c + 1) * QC]                      # [D, QC]
        in_maps.append({
            "xT": xT_b,
            "wqT": np.ascontiguousarray(wq_c.T).astype(bf16),
            "wkvT": np.ascontiguousarray(wkv_c.T).astype(bf16),
            "woT": np.ascontiguousarray(wo_c.T).astype(bf16),
            "cosT": cosT.astype(bf16),
            "sinT": sinT.astype(bf16),
        })
    return in_maps


def kernel(x, wq, wk, wv, wo):
    from concourse.bass_utils import run_bass_kernel_spmd

    nc = _get_nc()
    in_maps = _prep_inputs(x, wq, wk, wv, wo)
    res = run_bass_kernel_spmd(nc, in_maps, core_ids=list(range(NCORES)))
    acc = np.zeros((T, D), dtype=np.float64)
    for c in range(NCORES):
        acc += res.results[c]["out"].astype(np.float64)
    return acc.astype(np.float32).reshape(1, T, D)
